# revision 1
# baseline (speedup 1.0000x reference)
"""DGI (Deep Graph Infomax) Trainium2 kernel.

Strategy (8 NeuronCores, one shared SPMD program):
  - Nodes sharded by destination: core c owns dst nodes [c*N/8, (c+1)*N/8).
  - xw = x @ W computed replicated on every core in fp16 (DMA-transpose
    loads + PE matmuls), written to per-core DRAM.
  - GCN aggregation: edges (incl. self-loops, symmetric norm precomputed on
    host) are sorted by (dst tile, src<32768), padded to 128-edge tiles with
    (idx=0, w=0, dstl=-1).  Each 128-edge tile: dma_gather of xw rows
    (int16 indices; hi half gathered from an offset AP), weighted one-hot
    S_Tw built on DVE (is_equal vs iota, scaled by norm), PE matmul
    S_Tw.T @ gathered accumulated into the dst tile's PSUM.
  - PReLU(agg + b) -> z tiles; z1 kept in SBUF, z2 streamed.
  - summary = sigmoid(mean(z1)): DVE tree column-sum + ones-matmul,
    1KB AllReduce across the 8 cores, sigmoid on ACT.
  - wsum = disc_W @ summary via PE (host passes disc_W.T); broadcast via
    K=1 matmul; pos/neg = z . wsum via fused tensor_tensor_reduce.
  - Per-core [128, DT] outputs; host unshards/concatenates.
"""

import os

import numpy as np

_P = 128
_LO = 32768
_C = 8


def _build_streams(sidx, ed, ew, C, NS, DT):
    """Build per-core gather/weight/dstl streams with a shared tile structure.

    sidx: source index per edge (already permuted for the corrupted pass)
    ed:   destination node per edge
    ew:   edge weight (symmetric norm) per edge
    Returns (idx_sbuf [C,128,n_et*8] i16, w_sbuf [C,128,n_et] f32,
             dl_sbuf [C,128,n_et] f16, Tmax [DT,2] int, off_tiles [DT,2] int,
             n_et)
    """
    core = ed // NS
    ldst = ed - core * NS
    dt = ldst // _P
    dstl = ldst % _P
    cls = (sidx >= _LO).astype(np.int64)

    gid = (core * DT + dt) * 2 + cls
    NG = C * DT * 2
    cnt = np.bincount(gid, minlength=NG).reshape(C, DT, 2)
    T = -(-cnt // _P)
    Tmax = T.max(axis=0)  # shared structure across cores
    flat = Tmax.reshape(-1)
    off_tiles = np.concatenate([[0], np.cumsum(flat)[:-1]]).reshape(DT, 2)
    n_et = int(flat.sum())

    order = np.argsort(gid, kind="stable")
    sorted_gid = gid[order]
    g_starts = np.concatenate(
        [[0], np.cumsum(np.bincount(sorted_gid, minlength=NG))[:-1]]
    )
    rank = np.arange(order.size) - g_starts[sorted_gid]
    g_dt = (sorted_gid // 2) % DT
    g_cls = sorted_gid % 2
    pos = off_tiles[g_dt, g_cls] * _P + rank
    core_s = sorted_gid // (DT * 2)

    L = n_et * _P
    idx16 = np.zeros((C, L), np.int16)
    wv = np.zeros((C, L), np.float32)
    dl = np.full((C, L), -1.0, np.float16)
    sidx_s = sidx[order]
    idx16[core_s, pos] = (sidx_s - g_cls * _LO).astype(np.int16)
    wv[core_s, pos] = ew[order]
    dl[core_s, pos] = dstl[order].astype(np.float16)

    idx_w = idx16.reshape(C, L // 16, 16).transpose(0, 2, 1)
    idx_sbuf = np.ascontiguousarray(np.tile(idx_w, (1, 8, 1)))
    w_sbuf = np.ascontiguousarray(wv.reshape(C, n_et, _P).transpose(0, 2, 1))
    dl_sbuf = np.ascontiguousarray(dl.reshape(C, n_et, _P).transpose(0, 2, 1))
    return idx_sbuf, w_sbuf, dl_sbuf, Tmax, off_tiles, n_et


def kernel(x, W, b, a, disc_W, edge_index, perm):
    import bass_rust
    import concourse.bacc as bacc
    import concourse.mybir as mybir
    import concourse.tile as tile
    from concourse.bass_utils import run_bass_kernel_spmd

    x = np.asarray(x)
    W = np.asarray(W)
    b = np.asarray(b, np.float32)
    a = np.asarray(a, np.float32)
    disc_W = np.asarray(disc_W, np.float32)
    ei = np.asarray(edge_index, np.int64)
    perm_np = np.asarray(perm, np.int64)

    N, F = x.shape
    H = W.shape[1]
    C = _C
    NS = N // C
    DT = -(-NS // _P)
    LAST = NS - (DT - 1) * _P  # valid rows of the last dst tile
    f16 = mybir.dt.float16
    f32 = mybir.dt.float32

    # ---- host preprocessing -------------------------------------------
    src = ei[0]
    dst = ei[1]
    deg = (np.bincount(dst, minlength=N) + 1.0).astype(np.float32)
    dinv = (1.0 / np.sqrt(deg)).astype(np.float32)
    loops = np.arange(N, dtype=np.int64)
    es = np.concatenate([src, loops])
    ed = np.concatenate([dst, loops])
    ew = dinv[es] * dinv[ed]
    es2 = perm_np[es]

    i1, w1, d1, T1, O1, n_et1 = _build_streams(es, ed, ew, C, NS, DT)
    i2, w2, d2, T2, O2, n_et2 = _build_streams(es2, ed, ew, C, NS, DT)

    x_f16 = np.ascontiguousarray(x.astype(np.float16))
    W_f16 = np.ascontiguousarray(W.astype(np.float16))
    dwT = np.ascontiguousarray(disc_W.T.astype(np.float32))
    iota_np = np.tile(np.arange(_P, dtype=np.float16)[None, :], (_P, 1))

    # ---- device program -----------------------------------------------
    nc = bacc.Bacc("TRN2", target_bir_lowering=False, debug=False, num_devices=C)

    t_x = nc.dram_tensor("x16", [N, F], f16, kind="ExternalInput")
    t_W = nc.dram_tensor("w16", [F, H], f16, kind="ExternalInput")
    t_b = nc.dram_tensor("bvec", [H], f32, kind="ExternalInput")
    t_a = nc.dram_tensor("avec", [1], f32, kind="ExternalInput")
    t_dwT = nc.dram_tensor("dwT", [H, H], f32, kind="ExternalInput")
    t_iota = nc.dram_tensor("iota", [_P, _P], f16, kind="ExternalInput")
    t_ident = nc.dram_tensor("ident_in", [_P, _P], f32, kind="ExternalInput")
    t_i1 = nc.dram_tensor("idx1", [_P, n_et1 * 8], mybir.dt.int16, kind="ExternalInput")
    t_w1 = nc.dram_tensor("wgt1", [_P, n_et1], f32, kind="ExternalInput")
    t_d1 = nc.dram_tensor("dstl1", [_P, n_et1], f16, kind="ExternalInput")
    t_i2 = nc.dram_tensor("idx2", [_P, n_et2 * 8], mybir.dt.int16, kind="ExternalInput")
    t_w2 = nc.dram_tensor("wgt2", [_P, n_et2], f32, kind="ExternalInput")
    t_d2 = nc.dram_tensor("dstl2", [_P, n_et2], f16, kind="ExternalInput")

    t_pos = nc.dram_tensor("pos_out", [_P, DT], f32, kind="ExternalOutput")
    t_neg = nc.dram_tensor("neg_out", [_P, DT], f32, kind="ExternalOutput")

    t_xw = nc.dram_tensor("xw", [N, H], f16)
    t_ar_in = nc.dram_tensor("ar_in", [H], f32)
    t_ar_out = nc.dram_tensor("ar_out", [H], f32, addr_space="Shared")

    CHUNK = 512  # phase-1 node rows per transposed load
    STAGE = int(os.environ.get("KERNEL_STAGE", "4"))

    with tile.TileContext(nc) as tc:
        import contextlib

        ctx = contextlib.ExitStack()
        consts = ctx.enter_context(tc.tile_pool(name="consts", bufs=1))
        ph1 = ctx.enter_context(tc.tile_pool(name="ph1", bufs=3))
        ph1ps = ctx.enter_context(tc.tile_pool(name="ph1ps", bufs=2, space="PSUM"))
        glo = ctx.enter_context(tc.tile_pool(name="glo", bufs=2))
        ghi = ctx.enter_context(tc.tile_pool(name="ghi", bufs=2))
        stp = ctx.enter_context(tc.tile_pool(name="stp", bufs=4))
        aggps = ctx.enter_context(tc.tile_pool(name="aggps", bufs=3, space="PSUM"))
        misc = ctx.enter_context(tc.tile_pool(name="misc", bufs=2))
        miscps = ctx.enter_context(tc.tile_pool(name="miscps", bufs=1, space="PSUM"))

        # ---- constants ----
        W0 = consts.tile([_P, H], f16, tag="W0")
        W1 = consts.tile([_P, H], f16, tag="W1")
        nc.sync.dma_start(W0[:], t_W[0:_P, :])
        nc.sync.dma_start(W1[:], t_W[_P : 2 * _P, :])
        iota_t = consts.tile([_P, _P], f16, tag="iota")
        nc.sync.dma_start(iota_t[:], t_iota[:])
        b_sb = consts.tile([1, H], f32, tag="b_sb")
        nc.sync.dma_start(b_sb[:], t_b[None, :])
        a_sb = consts.tile([1, 1], f32, tag="a_sb")
        nc.sync.dma_start(a_sb[:], t_a[None, :])
        dwT0 = consts.tile([_P, H], f32, tag="dwT0")
        dwT1 = consts.tile([_P, H], f32, tag="dwT1")
        nc.sync.dma_start(dwT0[:], t_dwT[0:_P, :])
        nc.sync.dma_start(dwT1[:], t_dwT[_P : 2 * _P, :])
        ones_row = consts.tile([1, _P], f32, tag="ones_row")
        nc.vector.memset(ones_row[:], 1.0)
        ones_col = consts.tile([_P, 1], f32, tag="ones_col")
        nc.vector.memset(ones_col[:], 1.0)

        # broadcasts via K=1 matmul
        bb_ps = miscps.tile([_P, H], f32, tag="mps")
        nc.tensor.matmul(bb_ps[:], ones_row[:], b_sb[:], start=True, stop=True)
        b_bc = consts.tile([_P, H], f32, tag="b_bc")
        nc.vector.tensor_copy(b_bc[:], bb_ps[:])
        ab_ps = miscps.tile([_P, 1], f32, tag="mps")
        nc.tensor.matmul(ab_ps[:], ones_row[:], a_sb[:], start=True, stop=True)
        a_bc = consts.tile([_P, 1], f32, tag="a_bc")
        nc.vector.tensor_copy(a_bc[:], ab_ps[:])

        # ---- stream loads ----
        i1_sb = consts.tile([_P, n_et1 * 8], mybir.dt.int16, tag="i1")
        w1_sb = consts.tile([_P, n_et1], f32, tag="w1")
        d1_sb = consts.tile([_P, n_et1], f16, tag="d1")
        nc.sync.dma_start(i1_sb[:], t_i1[:])
        nc.sync.dma_start(w1_sb[:], t_w1[:])
        nc.sync.dma_start(d1_sb[:], t_d1[:])
        i2_sb = consts.tile([_P, n_et2 * 8], mybir.dt.int16, tag="i2")
        w2_sb = consts.tile([_P, n_et2], f32, tag="w2")
        d2_sb = consts.tile([_P, n_et2], f16, tag="d2")
        nc.sync.dma_start(i2_sb[:], t_i2[:])
        nc.sync.dma_start(w2_sb[:], t_w2[:])
        nc.sync.dma_start(d2_sb[:], t_d2[:])

        # ---- phase 1: xw = x @ W (replicated) ----
        for r0 in range(0, N, CHUNK):
            rows = min(CHUNK, N - r0)
            xT0 = ph1.tile([_P, CHUNK], f16, tag="xT0")
            xT1 = ph1.tile([_P, CHUNK], f16, tag="xT1")
            nc.sync.dma_start_transpose(xT0[:, :rows], t_x[r0 : r0 + rows, 0:_P])
            nc.sync.dma_start_transpose(
                xT1[:, :rows], t_x[r0 : r0 + rows, _P : 2 * _P]
            )
            for o in range(0, rows, _P):
                m = min(_P, rows - o)
                ps = ph1ps.tile([_P, H], f32, tag="ph1ps")
                nc.tensor.matmul(
                    ps[:m, :], xT0[:, o : o + m], W0[:], start=True, stop=False
                )
                nc.tensor.matmul(
                    ps[:m, :], xT1[:, o : o + m], W1[:], start=False, stop=True
                )
                xw_sb = ph1.tile([_P, H], f16, tag="xw_sb")
                nc.any.tensor_copy(xw_sb[:m, :], ps[:m, :])
                nc.sync.dma_start(t_xw[r0 + o : r0 + o + m, :], xw_sb[:m, :])

        xw_lo = t_xw[0:_LO, :] if N > _LO else t_xw[:, :]
        xw_hi = t_xw[_LO:N, :] if N > _LO else None

        # ---- aggregation passes ----
        zbuf = consts.tile([_P, DT * H], f32, tag="zbuf")  # z1 persists

        def agg_pass(idx_sb, w_sb, dl_sb, Tm, Ot, z_consumer):
            for dti in range(DT):
                Tl, Th = int(Tm[dti, 0]), int(Tm[dti, 1])
                gl = gh = None
                if Tl:
                    o = int(Ot[dti, 0])
                    gl = glo.tile([_P, max_Tl, H], f16, tag="gl")
                    nc.gpsimd.dma_gather(
                        gl[:, :Tl, :],
                        xw_lo,
                        idx_sb[:, 8 * o : 8 * (o + Tl)],
                        Tl * _P,
                        Tl * _P,
                        H,
                        single_packet=(Tl * _P <= 1024),
                    )
                if Th:
                    o = int(Ot[dti, 1])
                    gh = ghi.tile([_P, max_Th, H], f16, tag="gh")
                    nc.gpsimd.dma_gather(
                        gh[:, :Th, :],
                        xw_hi,
                        idx_sb[:, 8 * o : 8 * (o + Th)],
                        Th * _P,
                        Th * _P,
                        H,
                        single_packet=(Th * _P <= 1024),
                    )
                ps = aggps.tile([_P, H], f32, tag="aggps")
                n_mm = Tl + Th
                k = 0
                for cls_i, (Tn, g, o0) in enumerate(
                    [(Tl, gl, int(Ot[dti, 0])), (Th, gh, int(Ot[dti, 1]))]
                ):
                    for j in range(Tn):
                        t = o0 + j
                        eq = stp.tile([_P, _P], f16, tag="eq")
                        nc.vector.tensor_tensor(
                            eq[:],
                            dl_sb[:, t : t + 1].to_broadcast([_P, _P]),
                            iota_t[:],
                            mybir.AluOpType.is_equal,
                        )
                        stw = stp.tile([_P, _P], f16, tag="stw")
                        nc.vector.tensor_scalar(
                            stw[:],
                            eq[:],
                            w_sb[:, t : t + 1],
                            None,
                            mybir.AluOpType.mult,
                        )
                        nc.tensor.matmul(
                            ps[:],
                            stw[:],
                            g[:, j, :],
                            start=(k == 0),
                            stop=(k == n_mm - 1),
                        )
                        k += 1
                z_consumer(dti, ps)

        # z1 consumer: bias + PReLU into persistent zbuf
        def z1_consume(dti, ps):
            zs = zbuf[:, dti * H : (dti + 1) * H]
            nc.vector.tensor_tensor(zs, ps[:], b_bc[:], mybir.AluOpType.add)
            t1 = misc.tile([_P, H], f32, tag="t1")
            nc.vector.tensor_scalar(
                t1[:], zs, 0.0, a_bc[:, 0:1],
                mybir.AluOpType.min, mybir.AluOpType.mult,
            )
            t2 = misc.tile([_P, H], f32, tag="t2")
            nc.vector.tensor_scalar(t2[:], zs, 0.0, None, mybir.AluOpType.max)
            nc.vector.tensor_tensor(zs, t1[:], t2[:], mybir.AluOpType.add)

        max_Tl = max(int(T1[:, 0].max()), int(T2[:, 0].max()), 1)
        max_Th = max(int(T1[:, 1].max()), int(T2[:, 1].max()), 1)

        pos_acc = consts.tile([_P, DT], f32, tag="pos_acc")
        neg_acc = consts.tile([_P, DT], f32, tag="neg_acc")
        nc.vector.memset(pos_acc[:], 0.0)
        nc.vector.memset(neg_acc[:], 0.0)

        if STAGE >= 2:
            agg_pass(i1_sb, w1_sb, d1_sb, T1, O1, z1_consume)

        # ---- summary: column sum of z1 over all nodes ----
        if STAGE >= 3:
            cacc = misc.tile([_P, H], f32, tag="cacc")
            nc.vector.tensor_copy(cacc[:], zbuf[:, 0:H])
            for dti in range(1, DT):
                rows = LAST if dti == DT - 1 else _P
                nc.vector.tensor_tensor(
                    cacc[:rows, :],
                    cacc[:rows, :],
                    zbuf[:rows, dti * H : (dti + 1) * H],
                    mybir.AluOpType.add,
                )
            cs_ps = miscps.tile([1, H], f32, tag="mps")
            nc.tensor.matmul(cs_ps[:], ones_col[:], cacc[:], start=True, stop=True)
            cs_sb = misc.tile([1, H], f32, tag="cs_sb")
            nc.vector.tensor_copy(cs_sb[:], cs_ps[:])
            nc.sync.dma_start(t_ar_in[None, :], cs_sb[:])
            nc.gpsimd.collective_compute(
                "AllReduce",
                mybir.AluOpType.add,
                replica_groups=[list(range(C))],
                ins=[t_ar_in[:]],
                outs=[t_ar_out[:]],
            )
            sums_sb = misc.tile([1, H], f32, tag="sums_sb")
            nc.sync.dma_start(sums_sb[:], t_ar_out[None, :])
            summ_sb = misc.tile([1, H], f32, tag="summ_sb")
            nc.scalar.activation(
                summ_sb[:], sums_sb[:], mybir.ActivationFunctionType.Sigmoid,
                scale=1.0 / N,
            )

            # ---- wsum = disc_W @ summary ----
            ident = consts.tile([_P, _P], f32, tag="ident")
            nc.sync.dma_start(ident[:], t_ident[:])
            sT = misc.tile([_P, 2], f32, tag="sT")
            for c_i in range(2):
                tp = miscps.tile([_P, _P], f32, tag="mps")
                nc.tensor.transpose(
                    tp[:, 0:1],
                    summ_sb[0:1, c_i * _P : (c_i + 1) * _P],
                    ident[0:1, 0:1],
                )
                nc.vector.tensor_copy(sT[:, c_i : c_i + 1], tp[:, 0:1])
            ws_ps = miscps.tile([1, H], f32, tag="mps")
            nc.tensor.matmul(ws_ps[:], sT[:, 0:1], dwT0[:], start=True, stop=False)
            nc.tensor.matmul(ws_ps[:], sT[:, 1:2], dwT1[:], start=False, stop=True)
            ws_sb = misc.tile([1, H], f32, tag="ws_sb")
            nc.vector.tensor_copy(ws_sb[:], ws_ps[:])
            wb_ps = miscps.tile([_P, H], f32, tag="mps")
            nc.tensor.matmul(wb_ps[:], ones_row[:], ws_sb[:], start=True, stop=True)
            wsum_bc = consts.tile([_P, H], f32, tag="wsum_bc")
            nc.vector.tensor_copy(wsum_bc[:], wb_ps[:])

        scratch = misc.tile([_P, H], f32, tag="scratch")

        # ---- z2 pass with inline neg dot ----
        def z2_consume(dti, ps):
            zt = misc.tile([_P, H], f32, tag="zt")
            nc.vector.tensor_tensor(zt[:], ps[:], b_bc[:], mybir.AluOpType.add)
            t1 = misc.tile([_P, H], f32, tag="t1")
            nc.vector.tensor_scalar(
                t1[:], zt[:], 0.0, a_bc[:, 0:1],
                mybir.AluOpType.min, mybir.AluOpType.mult,
            )
            t2 = misc.tile([_P, H], f32, tag="t2")
            nc.vector.tensor_scalar(t2[:], zt[:], 0.0, None, mybir.AluOpType.max)
            nc.vector.tensor_tensor(zt[:], t1[:], t2[:], mybir.AluOpType.add)
            nc.vector.tensor_tensor(
                scratch[:], zt[:], wsum_bc[:], mybir.AluOpType.mult
            )
            nc.vector.reduce_sum(
                neg_acc[:, dti : dti + 1], scratch[:], bass_rust.AxisListType.X
            )

        if STAGE >= 4:
            agg_pass(i2_sb, w2_sb, d2_sb, T2, O2, z2_consume)

            # ---- pos dots from persistent z1 ----
            for dti in range(DT):
                nc.vector.tensor_tensor(
                    scratch[:], zbuf[:, dti * H : (dti + 1) * H], wsum_bc[:],
                    mybir.AluOpType.mult,
                )
                nc.vector.reduce_sum(
                    pos_acc[:, dti : dti + 1], scratch[:], bass_rust.AxisListType.X
                )

        nc.sync.dma_start(t_pos[:], pos_acc[:])
        nc.sync.dma_start(t_neg[:], neg_acc[:])
        ctx.close()

    nc.compile()

    in_maps = []
    for c in range(C):
        in_maps.append(
            {
                "x16": x_f16,
                "w16": W_f16,
                "bvec": b,
                "avec": a,
                "dwT": dwT,
                "iota": iota_np,
                "ident_in": np.eye(_P, dtype=np.float32),
                "idx1": i1[c],
                "wgt1": w1[c],
                "dstl1": d1[c],
                "idx2": i2[c],
                "wgt2": w2[c],
                "dstl2": d2[c],
            }
        )

    if os.environ.get("KERNEL_SIM", "0") == "1":
        from concourse import bass_interp

        sim = bass_interp.MultiCoreSim(nc, C)
        for c in range(C):
            for k, v in in_maps[c].items():
                sim.cores[c].tensor(k)[:] = v
        sim.simulate()
        results = [
            {
                "pos_out": np.array(sim.cores[c].tensor("pos_out")),
                "neg_out": np.array(sim.cores[c].tensor("neg_out")),
            }
            for c in range(C)
        ]
    else:
        trace = os.environ.get("KERNEL_TRACE", "0") == "1"
        kw = {}
        if trace:
            kw["trace"] = True
        res = run_bass_kernel_spmd(nc, in_maps, core_ids=list(range(C)), **kw)
        kernel.last_result = res
        results = res.results

    pos = np.zeros(N, np.float32)
    neg = np.zeros(N, np.float32)
    for c in range(C):
        pos[c * NS : (c + 1) * NS] = results[c]["pos_out"].T.reshape(-1)[:NS]
        neg[c * NS : (c + 1) * NS] = results[c]["neg_out"].T.reshape(-1)[:NS]
    return pos, neg



# revision 4
# speedup vs baseline: 2.1232x; 2.1232x over previous
"""DGI (Deep Graph Infomax) Trainium2 kernel — v2.

Strategy (8 NeuronCores, one shared SPMD program):
  - Host packs xc = [x | x[perm]] and uploads it TRANSPOSED (xcT [512, N] f16),
    so phase 1 needs no DMA transposes.
  - Phase 1 (replicated): xwc[v] = dinv[v] * [x[v]@W | x[perm[v]]@W] (f16,
    1024B rows) written to two DRAM tables split at node 17280 (=135*128) so
    low-class gathers can start while phase 1 finishes the high range.
    dinv[s] is folded here, making the aggregation one-hots BINARY.
  - Aggregation (dst-sharded, both passes fused): per (dst tile, class) ONE
    dma_gather of 1024B rows serves z1 AND z2.  Per 128-edge tile: binary
    one-hot via is_equal on DVE, ONE 128x128x512 PE matmul accumulating
    [z1|z2] into a single PSUM bank.  Two class rounds accumulate via SBUF.
  - Consumer: scale by dinv[d], (bias if nonzero), PReLU -> zacc in SBUF.
    z1 column sums accumulate in PSUM via ones-vector matmuls as tiles finish.
  - summary sigmoid + 1KB AllReduce + wsum = disc_W @ summary (PE), then
    pos/neg dots per dst tile on DVE.  Host unshards [128, 49] outputs.
"""

import os

import numpy as np

_P = 128
_C = 8
_SPLIT = 17280  # 135 * 128; hi span = 50000-17280 = 32720 <= 32768 (int16)


def _build_streams(es, ed, C, NS, DT):
    """Per-core gather index + local-dst streams with shared tile structure.

    Groups edges by (dst_tile, class) where class = src >= _SPLIT.
    Pads with idx=0 / dstl=-1 to Tmax (max tiles over cores) per group.
    Returns (idx_sbuf [C,128,n_et*8] i16, dl_sbuf [C,128,n_et] f16,
             Tmax [DT,2], off [DT,2], n_et)
    """
    core = ed // NS
    ldst = ed - core * NS
    dt = ldst // _P
    dstl = ldst - dt * _P
    cls = (es >= _SPLIT).astype(np.int64)

    gid = (core * DT + dt) * 2 + cls
    NG = C * DT * 2
    cnt = np.bincount(gid, minlength=NG).reshape(C, DT, 2)
    T = -(-cnt // _P)
    Tmax = np.maximum(T.max(axis=0), 1)
    flat = Tmax.reshape(-1)
    off = np.concatenate([[0], np.cumsum(flat)[:-1]]).reshape(DT, 2)
    n_et = int(flat.sum())

    order = np.argsort(gid, kind="stable")
    sorted_gid = gid[order]
    g_starts = np.concatenate(
        [[0], np.cumsum(np.bincount(sorted_gid, minlength=NG))[:-1]]
    )
    rank = np.arange(order.size) - g_starts[sorted_gid]
    g_dt = (sorted_gid // 2) % DT
    g_cls = sorted_gid % 2
    pos = off[g_dt, g_cls] * _P + rank
    core_s = sorted_gid // (DT * 2)

    L = n_et * _P
    idx16 = np.zeros((C, L), np.int16)
    dl = np.full((C, L), -1.0, np.float16)
    es_s = es[order]
    idx16[core_s, pos] = (es_s - g_cls * _SPLIT).astype(np.int16)
    dl[core_s, pos] = dstl[order].astype(np.float16)

    idx_w = idx16.reshape(C, L // 16, 16).transpose(0, 2, 1)
    idx_sbuf = np.ascontiguousarray(np.tile(idx_w, (1, 8, 1)))
    dl_sbuf = np.ascontiguousarray(dl.reshape(C, n_et, _P).transpose(0, 2, 1))
    return idx_sbuf, dl_sbuf, Tmax, off, n_et


def kernel(x, W, b, a, disc_W, edge_index, perm):
    import bass_rust
    import concourse.bacc as bacc
    import concourse.mybir as mybir
    import concourse.tile as tile
    from concourse.bass_utils import run_bass_kernel_spmd

    x = np.asarray(x)
    W = np.asarray(W)
    b = np.asarray(b, np.float32)
    a = np.asarray(a, np.float32)
    disc_W = np.asarray(disc_W, np.float32)
    ei = np.asarray(edge_index, np.int64)
    perm_np = np.asarray(perm, np.int64)

    N, F = x.shape
    H = W.shape[1]
    C = _C
    NS = N // C
    DT = -(-NS // _P)
    LAST = NS - (DT - 1) * _P
    NT = -(-N // _P)  # global node tiles (391)
    NLO = _SPLIT
    NHI = N - _SPLIT
    f16 = mybir.dt.float16
    f32 = mybir.dt.float32
    has_bias = bool(np.any(b))

    # ---- host preprocessing -------------------------------------------
    src = ei[0]
    dst = ei[1]
    deg = (np.bincount(dst, minlength=N) + 1.0).astype(np.float32)
    dinv = (1.0 / np.sqrt(deg)).astype(np.float32)
    loops = np.arange(N, dtype=np.int64)
    es = np.concatenate([src, loops])
    ed = np.concatenate([dst, loops])

    i_s, d_s, Tm, Ot, n_et = _build_streams(es, ed, C, NS, DT)
    maxT = int(Tm.max())

    xc = np.concatenate([x, x[perm_np]], axis=1).astype(np.float16)
    xcT = np.ascontiguousarray(xc.T)  # [2F, N]
    dwT = np.ascontiguousarray(disc_W.T.astype(np.float32))
    iota_np = np.tile(np.arange(_P, dtype=np.float16)[None, :], (_P, 1))

    # dinv in node-tile layout [128, NT] (pad 0)
    dinv_nt = np.zeros((_P, NT), np.float32)
    dinv_pad = np.zeros(NT * _P, np.float32)
    dinv_pad[:N] = dinv
    dinv_nt[:, :] = dinv_pad.reshape(NT, _P).T
    # per-core dst-tile layout [128, DT] (pad 0)
    dinv_dst = np.zeros((C, _P, DT), np.float32)
    for c in range(C):
        dp = np.zeros(DT * _P, np.float32)
        dp[:NS] = dinv[c * NS : (c + 1) * NS]
        dinv_dst[c] = dp.reshape(DT, _P).T

    # ---- device program -----------------------------------------------
    nc = bacc.Bacc("TRN2", target_bir_lowering=False, debug=False, num_devices=C)

    t_xcT = nc.dram_tensor("xcT", [2 * F, N], f16, kind="ExternalInput")
    t_W = nc.dram_tensor("w32", [F, H], f32, kind="ExternalInput")
    t_b = nc.dram_tensor("bvec", [H], f32, kind="ExternalInput")
    t_a = nc.dram_tensor("avec", [1], f32, kind="ExternalInput")
    t_dwT = nc.dram_tensor("dwT", [H, H], f32, kind="ExternalInput")
    t_iota = nc.dram_tensor("iota", [_P, _P], f16, kind="ExternalInput")
    t_ident = nc.dram_tensor("ident_in", [_P, _P], f32, kind="ExternalInput")
    t_dnt = nc.dram_tensor("dinv_nt", [_P, NT], f32, kind="ExternalInput")
    t_ddst = nc.dram_tensor("dinv_dst", [_P, DT], f32, kind="ExternalInput")
    t_i = nc.dram_tensor("idx", [_P, n_et * 8], mybir.dt.int16, kind="ExternalInput")
    t_d = nc.dram_tensor("dstl", [_P, n_et], f16, kind="ExternalInput")

    t_pos = nc.dram_tensor("pos_out", [_P, DT], f32, kind="ExternalOutput")
    t_neg = nc.dram_tensor("neg_out", [_P, DT], f32, kind="ExternalOutput")

    t_xwc_lo = nc.dram_tensor("xwc_lo", [NLO, 2 * H], f16)
    t_xwc_hi = nc.dram_tensor("xwc_hi", [NHI, 2 * H], f16)
    t_ar_in = nc.dram_tensor("ar_in", [H], f32)
    t_ar_out = nc.dram_tensor("ar_out", [H], f32, addr_space="Shared")

    CHUNK = 512

    with tile.TileContext(nc) as tc:
        import contextlib

        ctx = contextlib.ExitStack()
        consts = ctx.enter_context(tc.tile_pool(name="consts", bufs=1))
        ph1 = ctx.enter_context(tc.tile_pool(name="ph1", bufs=3))
        ph1ps = ctx.enter_context(tc.tile_pool(name="ph1ps", bufs=2, space="PSUM"))
        gpool = ctx.enter_context(tc.tile_pool(name="gpool", bufs=3))
        stp = ctx.enter_context(tc.tile_pool(name="stp", bufs=8))
        aggps = ctx.enter_context(tc.tile_pool(name="aggps", bufs=2, space="PSUM"))
        misc = ctx.enter_context(tc.tile_pool(name="misc", bufs=2))
        miscps = ctx.enter_context(tc.tile_pool(name="miscps", bufs=1, space="PSUM"))
        sumps = ctx.enter_context(tc.tile_pool(name="sumps", bufs=1, space="PSUM"))

        # ---- constants ----
        W0 = consts.tile([_P, H], f16, tag="W0")
        W1 = consts.tile([_P, H], f16, tag="W1")
        W0f = consts.tile([_P, H], f32, tag="W0f")
        W1f = consts.tile([_P, H], f32, tag="W1f")
        nc.sync.dma_start(W0f[:], t_W[0:_P, :])
        nc.sync.dma_start(W1f[:], t_W[_P : 2 * _P, :])
        nc.vector.tensor_copy(W0[:], W0f[:])
        nc.vector.tensor_copy(W1[:], W1f[:])
        iota_t = consts.tile([_P, _P], f16, tag="iota")
        nc.sync.dma_start(iota_t[:], t_iota[:])
        b_sb = consts.tile([1, H], f32, tag="b_sb")
        nc.sync.dma_start(b_sb[:], t_b[None, :])
        a_sb = consts.tile([1, 1], f32, tag="a_sb")
        nc.sync.dma_start(a_sb[:], t_a[None, :])
        dwT0 = consts.tile([_P, H], f32, tag="dwT0")
        dwT1 = consts.tile([_P, H], f32, tag="dwT1")
        nc.sync.dma_start(dwT0[:], t_dwT[0:_P, :])
        nc.sync.dma_start(dwT1[:], t_dwT[_P : 2 * _P, :])
        dnt = consts.tile([_P, NT], f32, tag="dnt")
        nc.sync.dma_start(dnt[:], t_dnt[:])
        ddst = consts.tile([_P, DT], f32, tag="ddst")
        nc.sync.dma_start(ddst[:], t_ddst[:])
        ones_row = consts.tile([1, _P], f32, tag="ones_row")
        nc.vector.memset(ones_row[:], 1.0)
        ones_col = consts.tile([_P, 1], f32, tag="ones_col")
        nc.vector.memset(ones_col[:], 1.0)

        # broadcasts via K=1 matmul
        bb_ps = miscps.tile([_P, H], f32, tag="mps")
        nc.tensor.matmul(bb_ps[:], ones_row[:], b_sb[:], start=True, stop=True)
        b_bc = consts.tile([_P, H], f32, tag="b_bc")
        nc.vector.tensor_copy(b_bc[:], bb_ps[:])
        ab_ps = miscps.tile([_P, 1], f32, tag="mps")
        nc.tensor.matmul(ab_ps[:], ones_row[:], a_sb[:], start=True, stop=True)
        a_bc = consts.tile([_P, 1], f32, tag="a_bc")
        nc.vector.tensor_copy(a_bc[:], ab_ps[:])

        # ---- stream loads ----
        i_sb = consts.tile([_P, n_et * 8], mybir.dt.int16, tag="i_sb")
        d_sb = consts.tile([_P, n_et], f16, tag="d_sb")
        nc.sync.dma_start(i_sb[:], t_i[:])
        nc.sync.dma_start(d_sb[:], t_d[:])

        # ---- phase 1: xwc = dinv * [x@W | x[perm]@W], lo tiles first ----
        def phase1_range(t0, t1):
            for tt0 in range(t0, t1, CHUNK // _P):
                ntiles = min(CHUNK // _P, t1 - tt0)
                cols = min(CHUNK, N - tt0 * _P)
                xt = []
                for fb in range(4):
                    xtb = ph1.tile([_P, CHUNK], f16, tag=f"xt{fb}")
                    nc.sync.dma_start(
                        xtb[:, :cols],
                        t_xcT[fb * _P : (fb + 1) * _P, tt0 * _P : tt0 * _P + cols],
                    )
                    xt.append(xtb)
                for o in range(ntiles):
                    ti = tt0 + o
                    m = min(_P, N - ti * _P)
                    psA = ph1ps.tile([_P, H], f32, tag="psA")
                    psB = ph1ps.tile([_P, H], f32, tag="psB")
                    nc.tensor.matmul(
                        psA[:m, :], xt[0][:, o * _P : o * _P + m], W0[:],
                        start=True, stop=False,
                    )
                    nc.tensor.matmul(
                        psA[:m, :], xt[1][:, o * _P : o * _P + m], W1[:],
                        start=False, stop=True,
                    )
                    nc.tensor.matmul(
                        psB[:m, :], xt[2][:, o * _P : o * _P + m], W0[:],
                        start=True, stop=False,
                    )
                    nc.tensor.matmul(
                        psB[:m, :], xt[3][:, o * _P : o * _P + m], W1[:],
                        start=False, stop=True,
                    )
                    xwc_sb = ph1.tile([_P, 2 * H], f16, tag="xwc_sb")
                    nc.scalar.activation(
                        xwc_sb[:m, 0:H], psA[:m, :],
                        mybir.ActivationFunctionType.Copy,
                        scale=dnt[:m, ti : ti + 1],
                    )
                    nc.scalar.activation(
                        xwc_sb[:m, H : 2 * H], psB[:m, :],
                        mybir.ActivationFunctionType.Copy,
                        scale=dnt[:m, ti : ti + 1],
                    )
                    r0 = ti * _P
                    if r0 < NLO:
                        nc.sync.dma_start(t_xwc_lo[r0 : r0 + m, :], xwc_sb[:m, :])
                    else:
                        nc.sync.dma_start(
                            t_xwc_hi[r0 - NLO : r0 - NLO + m, :], xwc_sb[:m, :]
                        )

        TLO = NLO // _P  # 135
        phase1_range(0, TLO)
        phase1_range(TLO, NT)

        # ---- aggregation: two class rounds, fused z1|z2 ----
        zacc = consts.tile([_P, DT * 2 * H], f32, tag="zacc")
        pos_acc = consts.tile([_P, DT], f32, tag="pos_acc")
        neg_acc = consts.tile([_P, DT], f32, tag="neg_acc")

        cs_ps = sumps.tile([1, H], f32, tag="cs_ps")

        def consume(dti):
            zs = zacc[:, dti * 2 * H : (dti + 1) * 2 * H]
            nc.vector.tensor_scalar(
                zs, zs, ddst[:, dti : dti + 1], None, mybir.AluOpType.mult
            )
            if has_bias:
                for h in range(2):
                    nc.vector.tensor_tensor(
                        zs[:, h * H : (h + 1) * H],
                        zs[:, h * H : (h + 1) * H],
                        b_bc[:],
                        mybir.AluOpType.add,
                    )
            t1 = misc.tile([_P, 2 * H], f32, tag="t1")
            nc.vector.tensor_scalar(
                t1[:], zs, 0.0, a_bc[:, 0:1],
                mybir.AluOpType.min, mybir.AluOpType.mult,
            )
            t2 = misc.tile([_P, 2 * H], f32, tag="t2")
            nc.vector.tensor_scalar(t2[:], zs, 0.0, None, mybir.AluOpType.max)
            nc.vector.tensor_tensor(zs, t1[:], t2[:], mybir.AluOpType.add)
            # accumulate z1 column sums on PE (valid: pad rows are exact 0)
            nc.tensor.matmul(
                cs_ps[:], ones_col[:], zs[:, 0:H],
                start=(dti == 0), stop=(dti == DT - 1),
            )

        for rnd, (src_t, n_src) in enumerate([(t_xwc_lo, NLO), (t_xwc_hi, NHI)]):
            for dti in range(DT):
                T = int(Tm[dti, rnd])
                o = int(Ot[dti, rnd])
                g = gpool.tile([_P, maxT, 2 * H], f16, tag="g")
                nc.gpsimd.dma_gather(
                    g[:, :T, :],
                    src_t[:, :],
                    i_sb[:, 8 * o : 8 * (o + T)],
                    T * _P,
                    T * _P,
                    2 * H,
                    single_packet=(T * _P <= 1024),
                )
                ps = aggps.tile([_P, 2 * H], f32, tag="aggps")
                for j in range(T):
                    t = o + j
                    eq = stp.tile([_P, _P], f16, tag="eq")
                    nc.vector.tensor_tensor(
                        eq[:],
                        d_sb[:, t : t + 1].to_broadcast([_P, _P]),
                        iota_t[:],
                        mybir.AluOpType.is_equal,
                    )
                    nc.tensor.matmul(
                        ps[:], eq[:], g[:, j, :], start=(j == 0), stop=(j == T - 1)
                    )
                zs = zacc[:, dti * 2 * H : (dti + 1) * 2 * H]
                if rnd == 0:
                    nc.vector.tensor_copy(zs, ps[:])
                else:
                    nc.vector.tensor_tensor(zs, zs, ps[:], mybir.AluOpType.add)
                    consume(dti)

        # ---- summary: sigmoid(colsum(z1)/N), AllReduce ----
        cs_sb = misc.tile([1, H], f32, tag="cs_sb")
        nc.vector.tensor_copy(cs_sb[:], cs_ps[:])
        nc.sync.dma_start(t_ar_in[None, :], cs_sb[:])
        nc.gpsimd.collective_compute(
            "AllReduce",
            mybir.AluOpType.add,
            replica_groups=[list(range(C))],
            ins=[t_ar_in[:]],
            outs=[t_ar_out[:]],
        )
        sums_sb = misc.tile([1, H], f32, tag="sums_sb")
        nc.sync.dma_start(sums_sb[:], t_ar_out[None, :])
        summ_sb = misc.tile([1, H], f32, tag="summ_sb")
        nc.scalar.activation(
            summ_sb[:], sums_sb[:], mybir.ActivationFunctionType.Sigmoid,
            scale=1.0 / N,
        )

        # ---- wsum = disc_W @ summary ----
        ident = consts.tile([_P, _P], f32, tag="ident")
        nc.sync.dma_start(ident[:], t_ident[:])
        sT = misc.tile([_P, 2], f32, tag="sT")
        for c_i in range(2):
            tp = miscps.tile([_P, _P], f32, tag="mps")
            nc.tensor.transpose(
                tp[:, 0:1],
                summ_sb[0:1, c_i * _P : (c_i + 1) * _P],
                ident[0:1, 0:1],
            )
            nc.vector.tensor_copy(sT[:, c_i : c_i + 1], tp[:, 0:1])
        ws_ps = miscps.tile([1, H], f32, tag="mps")
        nc.tensor.matmul(ws_ps[:], sT[:, 0:1], dwT0[:], start=True, stop=False)
        nc.tensor.matmul(ws_ps[:], sT[:, 1:2], dwT1[:], start=False, stop=True)
        ws_sb = misc.tile([1, H], f32, tag="ws_sb")
        nc.vector.tensor_copy(ws_sb[:], ws_ps[:])
        wb_ps = miscps.tile([_P, H], f32, tag="mps")
        nc.tensor.matmul(wb_ps[:], ones_row[:], ws_sb[:], start=True, stop=True)
        wsum_bc = consts.tile([_P, H], f32, tag="wsum_bc")
        nc.vector.tensor_copy(wsum_bc[:], wb_ps[:])

        # ---- pos/neg dots ----
        scratch = misc.tile([_P, H], f32, tag="scratch")
        for dti in range(DT):
            for h, acc in ((0, pos_acc), (1, neg_acc)):
                nc.vector.tensor_tensor(
                    scratch[:],
                    zacc[:, dti * 2 * H + h * H : dti * 2 * H + (h + 1) * H],
                    wsum_bc[:],
                    mybir.AluOpType.mult,
                )
                nc.vector.reduce_sum(
                    acc[:, dti : dti + 1], scratch[:], bass_rust.AxisListType.X
                )

        nc.sync.dma_start(t_pos[:], pos_acc[:])
        nc.sync.dma_start(t_neg[:], neg_acc[:])
        ctx.close()

    nc.compile()

    in_maps = []
    for c in range(C):
        in_maps.append(
            {
                "xcT": xcT,
                "w32": W.astype(np.float32),
                "bvec": b,
                "avec": a,
                "dwT": dwT,
                "iota": iota_np,
                "ident_in": np.eye(_P, dtype=np.float32),
                "dinv_nt": dinv_nt,
                "dinv_dst": dinv_dst[c],
                "idx": i_s[c],
                "dstl": d_s[c],
            }
        )

    if os.environ.get("KERNEL_SIM", "0") == "1":
        from concourse import bass_interp

        sim = bass_interp.MultiCoreSim(nc, C)
        for c in range(C):
            for k, v in in_maps[c].items():
                sim.cores[c].tensor(k)[:] = v
        sim.simulate()
        results = [
            {
                "pos_out": np.array(sim.cores[c].tensor("pos_out")),
                "neg_out": np.array(sim.cores[c].tensor("neg_out")),
            }
            for c in range(C)
        ]
    else:
        trace = os.environ.get("KERNEL_TRACE", "0") == "1"
        kw = {}
        if trace:
            kw["trace"] = True
        res = run_bass_kernel_spmd(nc, in_maps, core_ids=list(range(C)), **kw)
        kernel.last_result = res
        results = res.results

    pos = np.zeros(N, np.float32)
    neg = np.zeros(N, np.float32)
    for c in range(C):
        pos[c * NS : (c + 1) * NS] = results[c]["pos_out"].T.reshape(-1)[:NS]
        neg[c * NS : (c + 1) * NS] = results[c]["neg_out"].T.reshape(-1)[:NS]
    return pos, neg


# revision 14
# speedup vs baseline: 2.1878x; 1.0304x over previous
"""DGI (Deep Graph Infomax) Trainium2 kernel — v2.

Strategy (8 NeuronCores, one shared SPMD program):
  - Host packs xc = [x | x[perm]] and uploads it TRANSPOSED (xcT [512, N] f16),
    so phase 1 needs no DMA transposes.
  - Phase 1 (replicated): xwc[v] = dinv[v] * [x[v]@W | x[perm[v]]@W] (f16,
    1024B rows) written to two DRAM tables split at node 17280 (=135*128) so
    low-class gathers can start while phase 1 finishes the high range.
    dinv[s] is folded here, making the aggregation one-hots BINARY.
  - Aggregation (dst-sharded, both passes fused): per (dst tile, class) ONE
    dma_gather of 1024B rows serves z1 AND z2.  Per 128-edge tile: binary
    one-hot via is_equal on DVE, ONE 128x128x512 PE matmul accumulating
    [z1|z2] into a single PSUM bank.  Two class rounds accumulate via SBUF.
  - Consumer: scale by dinv[d], (bias if nonzero), PReLU -> zacc in SBUF.
    z1 column sums accumulate in PSUM via ones-vector matmuls as tiles finish.
  - summary sigmoid + 1KB AllReduce + wsum = disc_W @ summary (PE), then
    pos/neg dots per dst tile on DVE.  Host unshards [128, 49] outputs.
"""

import os

import numpy as np

_P = 128
_C = 8
_SPLIT = 17280  # 135 * 128; hi span = 50000-17280 = 32720 <= 32768 (int16)


def _build_streams(es, ed, C, NS, DT):
    """Per-core gather index + local-dst streams with shared tile structure.

    Groups edges by (dst_tile, class) where class = src >= _SPLIT.
    Pads with idx=0 / dstl=-1 to Tmax (max tiles over cores) per group.
    Returns (idx_sbuf [C,128,n_et*8] i16, dl_sbuf [C,128,n_et] f16,
             Tmax [DT,2], off [DT,2], n_et)
    """
    core = ed // NS
    ldst = ed - core * NS
    dt = ldst // _P
    dstl = ldst - dt * _P
    cls = (es >= _SPLIT).astype(np.int64)

    gid = (core * DT + dt) * 2 + cls
    NG = C * DT * 2
    cnt = np.bincount(gid, minlength=NG).reshape(C, DT, 2)
    T = -(-cnt // _P)
    Tmax = np.maximum(T.max(axis=0), 1)
    Mv = np.maximum(cnt.max(axis=0), 1)  # shared valid count per group
    flat = Tmax.reshape(-1)
    off = np.concatenate([[0], np.cumsum(flat)[:-1]]).reshape(DT, 2)
    n_et = int(flat.sum())

    order = np.argsort(gid, kind="stable")
    sorted_gid = gid[order]
    g_starts = np.concatenate(
        [[0], np.cumsum(np.bincount(sorted_gid, minlength=NG))[:-1]]
    )
    rank = np.arange(order.size) - g_starts[sorted_gid]
    g_dt = (sorted_gid // 2) % DT
    g_cls = sorted_gid % 2
    pos = off[g_dt, g_cls] * _P + rank
    core_s = sorted_gid // (DT * 2)

    L = n_et * _P
    # idx=-1 (firmware skips trailing negatives); dummy-valid idx=0 fill up
    # to the shared per-group count Mv so num_idxs_reg is core-invariant.
    idx16 = np.full((C, L), -1, np.int16)
    dl = np.full((C, L), -1.0, np.float16)
    es_s = es[order]
    idx16[core_s, pos] = (es_s - g_cls * _SPLIT).astype(np.int16)
    dl[core_s, pos] = dstl[order].astype(np.float16)
    for dti in range(DT):
        for ci in range(2):
            base = off[dti, ci] * _P
            m = int(Mv[dti, ci])
            for c in range(C):
                v = int(cnt[c, dti, ci])
                if v < m:
                    idx16[c, base + v : base + m] = 0

    idx_w = idx16.reshape(C, L // 16, 16).transpose(0, 2, 1)
    idx_sbuf = np.ascontiguousarray(np.tile(idx_w, (1, 8, 1)))
    dl_sbuf = np.ascontiguousarray(dl.reshape(C, n_et, _P).transpose(0, 2, 1))
    return idx_sbuf, dl_sbuf, Tmax, Mv, off, n_et


def kernel(x, W, b, a, disc_W, edge_index, perm):
    import bass_rust
    import concourse.bacc as bacc
    import concourse.mybir as mybir
    import concourse.tile as tile
    from concourse.bass_utils import run_bass_kernel_spmd

    x = np.asarray(x)
    W = np.asarray(W)
    b = np.asarray(b, np.float32)
    a = np.asarray(a, np.float32)
    disc_W = np.asarray(disc_W, np.float32)
    ei = np.asarray(edge_index, np.int64)
    perm_np = np.asarray(perm, np.int64)

    N, F = x.shape
    H = W.shape[1]
    C = _C
    NS = N // C
    DT = -(-NS // _P)
    LAST = NS - (DT - 1) * _P
    NT = -(-N // _P)  # global node tiles (391)
    NLO = _SPLIT
    NHI = N - _SPLIT
    f16 = mybir.dt.float16
    f32 = mybir.dt.float32
    has_bias = bool(np.any(b))

    # ---- host preprocessing -------------------------------------------
    src = ei[0]
    dst = ei[1]
    deg = (np.bincount(dst, minlength=N) + 1.0).astype(np.float32)
    dinv = (1.0 / np.sqrt(deg)).astype(np.float32)
    loops = np.arange(N, dtype=np.int64)
    es = np.concatenate([src, loops])
    ed = np.concatenate([dst, loops])

    i_s, d_s, Tm, Mv, Ot, n_et = _build_streams(es, ed, C, NS, DT)
    maxT = int(Tm.max())
    a0 = float(a.reshape(-1)[0])

    xc = np.concatenate([x, x[perm_np]], axis=1).astype(np.float16)
    xcT = np.ascontiguousarray(xc.T)  # [2F, N]
    dwT = np.ascontiguousarray(disc_W.T.astype(np.float32))
    iota_np = np.tile(np.arange(_P, dtype=np.float16)[None, :], (_P, 1))

    # dinv in node-tile layout [128, NT] (pad 0)
    dinv_nt = np.zeros((_P, NT), np.float32)
    dinv_pad = np.zeros(NT * _P, np.float32)
    dinv_pad[:N] = dinv
    dinv_nt[:, :] = dinv_pad.reshape(NT, _P).T
    # per-core dst-tile layout [128, DT] (pad 0)
    dinv_dst = np.zeros((C, _P, DT), np.float32)
    for c in range(C):
        dp = np.zeros(DT * _P, np.float32)
        dp[:NS] = dinv[c * NS : (c + 1) * NS]
        dinv_dst[c] = dp.reshape(DT, _P).T
    adinv_dst = a0 * dinv_dst  # for fused c*PReLU(v) = (a*c)v + ((1-a)c)max(v,0)
    c1_dst = (1.0 - a0) * dinv_dst

    # ---- device program -----------------------------------------------
    nc = bacc.Bacc("TRN2", target_bir_lowering=False, debug=False, num_devices=C)

    t_xcT = nc.dram_tensor("xcT", [2 * F, N], f16, kind="ExternalInput")
    t_W = nc.dram_tensor("w32", [F, H], f32, kind="ExternalInput")
    t_b = nc.dram_tensor("bvec", [H], f32, kind="ExternalInput")
    t_a = nc.dram_tensor("avec", [1], f32, kind="ExternalInput")
    t_dwT = nc.dram_tensor("dwT", [H, H], f32, kind="ExternalInput")
    t_iota = nc.dram_tensor("iota", [_P, _P], f16, kind="ExternalInput")
    t_ident = nc.dram_tensor("ident_in", [_P, _P], f32, kind="ExternalInput")
    t_dnt = nc.dram_tensor("dinv_nt", [_P, NT], f32, kind="ExternalInput")
    t_ddst = nc.dram_tensor("dinv_dst", [_P, DT], f32, kind="ExternalInput")
    t_adinv = nc.dram_tensor("adinv_dst", [_P, DT], f32, kind="ExternalInput")
    t_c1 = nc.dram_tensor("c1_dst", [_P, DT], f32, kind="ExternalInput")
    t_i = nc.dram_tensor("idx", [_P, n_et * 8], mybir.dt.int16, kind="ExternalInput")
    t_d = nc.dram_tensor("dstl", [_P, n_et], f16, kind="ExternalInput")

    t_pos = nc.dram_tensor("pos_out", [_P, DT], f32, kind="ExternalOutput")
    t_neg = nc.dram_tensor("neg_out", [_P, DT], f32, kind="ExternalOutput")

    t_xwc_lo = nc.dram_tensor("xwc_lo", [NLO, 2 * H], f16)
    t_xwc_hi = nc.dram_tensor("xwc_hi", [NHI, 2 * H], f16)
    t_ar_in = nc.dram_tensor("ar_in", [H], f32)
    t_ar_out = nc.dram_tensor("ar_out", [H], f32, addr_space="Shared")

    CHUNK = 512

    with tile.TileContext(nc) as tc:
        import contextlib

        ctx = contextlib.ExitStack()
        consts = ctx.enter_context(tc.tile_pool(name="consts", bufs=1))
        ph1 = ctx.enter_context(tc.tile_pool(name="ph1", bufs=3))
        ph1ps = ctx.enter_context(tc.tile_pool(name="ph1ps", bufs=2, space="PSUM"))
        gpool = ctx.enter_context(tc.tile_pool(name="gpool", bufs=3))
        stp = ctx.enter_context(tc.tile_pool(name="stp", bufs=8))
        aggps = ctx.enter_context(tc.tile_pool(name="aggps", bufs=2, space="PSUM"))
        misc = ctx.enter_context(tc.tile_pool(name="misc", bufs=2))
        miscps = ctx.enter_context(tc.tile_pool(name="miscps", bufs=1, space="PSUM"))
        sumps = ctx.enter_context(tc.tile_pool(name="sumps", bufs=1, space="PSUM"))

        # ---- constants ----
        W0 = consts.tile([_P, H], f16, tag="W0")
        W1 = consts.tile([_P, H], f16, tag="W1")
        W0f = consts.tile([_P, H], f32, tag="W0f")
        W1f = consts.tile([_P, H], f32, tag="W1f")
        nc.sync.dma_start(W0f[:], t_W[0:_P, :])
        nc.sync.dma_start(W1f[:], t_W[_P : 2 * _P, :])
        nc.vector.tensor_copy(W0[:], W0f[:])
        nc.vector.tensor_copy(W1[:], W1f[:])
        iota_t = consts.tile([_P, _P], f16, tag="iota")
        nc.sync.dma_start(iota_t[:], t_iota[:])
        b_sb = consts.tile([1, H], f32, tag="b_sb")
        nc.sync.dma_start(b_sb[:], t_b[None, :])
        a_sb = consts.tile([1, 1], f32, tag="a_sb")
        nc.sync.dma_start(a_sb[:], t_a[None, :])
        dwT0 = consts.tile([_P, H], f32, tag="dwT0")
        dwT1 = consts.tile([_P, H], f32, tag="dwT1")
        nc.sync.dma_start(dwT0[:], t_dwT[0:_P, :])
        nc.sync.dma_start(dwT1[:], t_dwT[_P : 2 * _P, :])
        dnt = consts.tile([_P, NT], f32, tag="dnt")
        nc.sync.dma_start(dnt[:], t_dnt[:])
        ddst = consts.tile([_P, DT], f32, tag="ddst")
        nc.sync.dma_start(ddst[:], t_ddst[:])
        adinv = consts.tile([_P, DT], f32, tag="adinv")
        nc.sync.dma_start(adinv[:], t_adinv[:])
        c1t = consts.tile([_P, DT], f32, tag="c1t")
        nc.sync.dma_start(c1t[:], t_c1[:])
        ones_row = consts.tile([1, _P], f32, tag="ones_row")
        nc.vector.memset(ones_row[:], 1.0)
        ones_col = consts.tile([_P, 1], f32, tag="ones_col")
        nc.vector.memset(ones_col[:], 1.0)

        # broadcasts via K=1 matmul
        bb_ps = miscps.tile([_P, H], f32, tag="mps")
        nc.tensor.matmul(bb_ps[:], ones_row[:], b_sb[:], start=True, stop=True)
        b_bc = consts.tile([_P, H], f32, tag="b_bc")
        nc.vector.tensor_copy(b_bc[:], bb_ps[:])
        ab_ps = miscps.tile([_P, 1], f32, tag="mps")
        nc.tensor.matmul(ab_ps[:], ones_row[:], a_sb[:], start=True, stop=True)
        a_bc = consts.tile([_P, 1], f32, tag="a_bc")
        nc.vector.tensor_copy(a_bc[:], ab_ps[:])

        # ---- stream loads ----
        i_sb = consts.tile([_P, n_et * 8], mybir.dt.int16, tag="i_sb")
        d_sb = consts.tile([_P, n_et], f16, tag="d_sb")
        nc.sync.dma_start(i_sb[:], t_i[:])
        nc.sync.dma_start(d_sb[:], t_d[:])

        # ---- phase 1: xwc = dinv * [x@W | x[perm]@W], lo tiles first ----
        def phase1_range(t0, t1):
            for tt0 in range(t0, t1, CHUNK // _P):
                ntiles = min(CHUNK // _P, t1 - tt0)
                cols = min(CHUNK, N - tt0 * _P)
                xt = []
                for fb in range(4):
                    xtb = ph1.tile([_P, CHUNK], f16, tag=f"xt{fb}")
                    nc.sync.dma_start(
                        xtb[:, :cols],
                        t_xcT[fb * _P : (fb + 1) * _P, tt0 * _P : tt0 * _P + cols],
                    )
                    xt.append(xtb)
                for o in range(ntiles):
                    ti = tt0 + o
                    m = min(_P, N - ti * _P)
                    psA = ph1ps.tile([_P, H], f32, tag="psA")
                    psB = ph1ps.tile([_P, H], f32, tag="psB")
                    nc.tensor.matmul(
                        psA[:m, :], xt[0][:, o * _P : o * _P + m], W0[:],
                        start=True, stop=False,
                    )
                    nc.tensor.matmul(
                        psA[:m, :], xt[1][:, o * _P : o * _P + m], W1[:],
                        start=False, stop=True,
                    )
                    nc.tensor.matmul(
                        psB[:m, :], xt[2][:, o * _P : o * _P + m], W0[:],
                        start=True, stop=False,
                    )
                    nc.tensor.matmul(
                        psB[:m, :], xt[3][:, o * _P : o * _P + m], W1[:],
                        start=False, stop=True,
                    )
                    xwc_sb = ph1.tile([_P, 2 * H], f16, tag="xwc_sb")
                    nc.scalar.activation(
                        xwc_sb[:m, 0:H], psA[:m, :],
                        mybir.ActivationFunctionType.Copy,
                        scale=dnt[:m, ti : ti + 1],
                    )
                    nc.vector.tensor_scalar(
                        xwc_sb[:m, H : 2 * H], psB[:m, :],
                        dnt[:m, ti : ti + 1], None, mybir.AluOpType.mult,
                    )
                    r0 = ti * _P
                    if r0 < NLO:
                        nc.sync.dma_start(t_xwc_lo[r0 : r0 + m, :], xwc_sb[:m, :])
                    else:
                        nc.sync.dma_start(
                            t_xwc_hi[r0 - NLO : r0 - NLO + m, :], xwc_sb[:m, :]
                        )

        TLO = NLO // _P  # 135
        phase1_range(0, TLO)
        phase1_range(TLO, NT)

        # ---- aggregation: two class rounds, fused z1|z2 ----
        zacc = consts.tile([_P, DT * 2 * H], f32, tag="zacc")
        pos_acc = consts.tile([_P, DT], f32, tag="pos_acc")
        neg_acc = consts.tile([_P, DT], f32, tag="neg_acc")

        cs_ps = sumps.tile([1, H], f32, tag="cs_ps")

        def consume(dti, ps):
            # raw = zacc(lo round) + ps(hi round); z = dinv_d * PReLU(raw)
            #     = (a*dinv_d)*raw + ((1-a)*dinv_d)*max(raw, 0)
            zs = zacc[:, dti * 2 * H : (dti + 1) * 2 * H]
            t0 = misc.tile([_P, 2 * H], f32, tag="t0")
            nc.vector.tensor_tensor(t0[:], zs, ps[:], mybir.AluOpType.add)
            if has_bias:
                nc.vector.tensor_scalar(
                    t0[:], t0[:], ddst[:, dti : dti + 1], None,
                    mybir.AluOpType.mult,
                )
                for h in range(2):
                    nc.vector.tensor_tensor(
                        t0[:, h * H : (h + 1) * H],
                        t0[:, h * H : (h + 1) * H],
                        b_bc[:],
                        mybir.AluOpType.add,
                    )
                t1 = misc.tile([_P, 2 * H], f32, tag="t1")
                nc.vector.tensor_scalar(
                    t1[:], t0[:], 0.0, a_bc[:, 0:1],
                    mybir.AluOpType.min, mybir.AluOpType.mult,
                )
                t2 = misc.tile([_P, 2 * H], f32, tag="t2")
                nc.vector.tensor_scalar(
                    t2[:], t0[:], 0.0, None, mybir.AluOpType.max
                )
                nc.vector.tensor_tensor(zs, t1[:], t2[:], mybir.AluOpType.add)
            else:
                t2 = misc.tile([_P, 2 * H], f32, tag="t2")
                nc.vector.tensor_scalar(
                    t2[:], t0[:], 0.0, c1t[:, dti : dti + 1],
                    mybir.AluOpType.max, mybir.AluOpType.mult,
                )
                nc.vector.scalar_tensor_tensor(
                    zs, t0[:], adinv[:, dti : dti + 1], t2[:],
                    mybir.AluOpType.mult, mybir.AluOpType.add,
                )
            # accumulate z1 column sums on PE (valid: pad rows are exact 0)
            nc.tensor.matmul(
                cs_ps[:], ones_col[:], zs[:, 0:H],
                start=(dti == 0), stop=(dti == DT - 1),
            )

        # memset gather ring bufs once: slots skipped by trailing -1 indices
        # stay stale; finite data keeps 0*garbage = 0 in PSUM.
        for _ in range(3):
            gz = gpool.tile([_P, maxT, 2 * H], f16, tag="g")
            nc.vector.memset(gz[:, :, :], 0.0)

        for rnd, src_t in enumerate([t_xwc_lo, t_xwc_hi]):
            for dti in range(DT):
                T = int(Tm[dti, rnd])
                o = int(Ot[dti, rnd])
                g = gpool.tile([_P, maxT, 2 * H], f16, tag="g")
                nc.gpsimd.dma_gather(
                    g[:, :T, :],
                    src_t[:, :],
                    i_sb[:, 8 * o : 8 * (o + T)],
                    T * _P,
                    int(Mv[dti, rnd]),
                    2 * H,
                    single_packet=(T * _P <= 1024),
                )
                ps = aggps.tile([_P, 2 * H], f32, tag="aggps")
                for j in range(T):
                    t = o + j
                    eq = stp.tile([_P, _P], f16, tag="eq")
                    nc.vector.tensor_tensor(
                        eq[:],
                        d_sb[:, t : t + 1].to_broadcast([_P, _P]),
                        iota_t[:],
                        mybir.AluOpType.is_equal,
                    )
                    nc.tensor.matmul(
                        ps[:], eq[:], g[:, j, :], start=(j == 0), stop=(j == T - 1)
                    )
                zs = zacc[:, dti * 2 * H : (dti + 1) * 2 * H]
                if rnd == 0:
                    nc.vector.tensor_copy(zs, ps[:])
                else:
                    consume(dti, ps)

        # ---- summary: sigmoid(colsum(z1)/N), AllReduce ----
        cs_sb = misc.tile([1, H], f32, tag="cs_sb")
        nc.vector.tensor_copy(cs_sb[:], cs_ps[:])
        nc.sync.dma_start(t_ar_in[None, :], cs_sb[:])
        nc.gpsimd.collective_compute(
            "AllReduce",
            mybir.AluOpType.add,
            replica_groups=[list(range(C))],
            ins=[t_ar_in[:]],
            outs=[t_ar_out[:]],
        )
        sums_sb = misc.tile([1, H], f32, tag="sums_sb")
        nc.sync.dma_start(sums_sb[:], t_ar_out[None, :])
        summ_sb = misc.tile([1, H], f32, tag="summ_sb")
        nc.scalar.activation(
            summ_sb[:], sums_sb[:], mybir.ActivationFunctionType.Sigmoid,
            scale=1.0 / N,
        )

        # ---- wsum = disc_W @ summary ----
        ident = consts.tile([_P, _P], f32, tag="ident")
        nc.sync.dma_start(ident[:], t_ident[:])
        sT = misc.tile([_P, 2], f32, tag="sT")
        for c_i in range(2):
            tp = miscps.tile([_P, _P], f32, tag="mps")
            nc.tensor.transpose(
                tp[:, 0:1],
                summ_sb[0:1, c_i * _P : (c_i + 1) * _P],
                ident[0:1, 0:1],
            )
            nc.vector.tensor_copy(sT[:, c_i : c_i + 1], tp[:, 0:1])
        ws_ps = miscps.tile([1, H], f32, tag="mps")
        nc.tensor.matmul(ws_ps[:], sT[:, 0:1], dwT0[:], start=True, stop=False)
        nc.tensor.matmul(ws_ps[:], sT[:, 1:2], dwT1[:], start=False, stop=True)
        ws_sb = misc.tile([1, H], f32, tag="ws_sb")
        nc.vector.tensor_copy(ws_sb[:], ws_ps[:])
        wb_ps = miscps.tile([_P, H], f32, tag="mps")
        nc.tensor.matmul(wb_ps[:], ones_row[:], ws_sb[:], start=True, stop=True)
        wsum_bc = consts.tile([_P, H], f32, tag="wsum_bc")
        nc.vector.tensor_copy(wsum_bc[:], wb_ps[:])

        # ---- pos/neg dots (fused multiply + row-sum) ----
        for dti in range(DT):
            for h, acc in ((0, pos_acc), (1, neg_acc)):
                scratch = misc.tile([_P, H], f32, tag="scratch")
                nc.vector.scalar_tensor_tensor(
                    scratch[:],
                    zacc[:, dti * 2 * H + h * H : dti * 2 * H + (h + 1) * H],
                    1.0,
                    wsum_bc[:],
                    mybir.AluOpType.mult,
                    mybir.AluOpType.mult,
                    accum_out=acc[:, dti : dti + 1],
                )

        nc.sync.dma_start(t_pos[:], pos_acc[:])
        nc.sync.dma_start(t_neg[:], neg_acc[:])
        ctx.close()

    nc.compile()

    in_maps = []
    for c in range(C):
        in_maps.append(
            {
                "xcT": xcT,
                "w32": W.astype(np.float32),
                "bvec": b,
                "avec": a,
                "dwT": dwT,
                "iota": iota_np,
                "ident_in": np.eye(_P, dtype=np.float32),
                "dinv_nt": dinv_nt,
                "dinv_dst": dinv_dst[c],
                "adinv_dst": adinv_dst[c],
                "c1_dst": c1_dst[c],
                "idx": i_s[c],
                "dstl": d_s[c],
            }
        )

    if os.environ.get("KERNEL_SIM", "0") == "1":
        from concourse import bass_interp

        sim = bass_interp.MultiCoreSim(nc, C)
        for c in range(C):
            for k, v in in_maps[c].items():
                sim.cores[c].tensor(k)[:] = v
        sim.simulate()
        results = [
            {
                "pos_out": np.array(sim.cores[c].tensor("pos_out")),
                "neg_out": np.array(sim.cores[c].tensor("neg_out")),
            }
            for c in range(C)
        ]
    else:
        trace = os.environ.get("KERNEL_TRACE", "0") == "1"
        kw = {}
        if trace:
            kw["trace"] = True
        res = run_bass_kernel_spmd(nc, in_maps, core_ids=list(range(C)), **kw)
        kernel.last_result = res
        results = res.results

    pos = np.zeros(N, np.float32)
    neg = np.zeros(N, np.float32)
    for c in range(C):
        pos[c * NS : (c + 1) * NS] = results[c]["pos_out"].T.reshape(-1)[:NS]
        neg[c * NS : (c + 1) * NS] = results[c]["neg_out"].T.reshape(-1)[:NS]
    return pos, neg
